# revision 16
# baseline (speedup 1.0000x reference)
"""Multi-head attention (B=4, S=1500, D=1024, H=16) on 8 TRN2 NeuronCores.

Sharding: (batch, head-half) -> core c = 2*b + h; each core computes the
full attention for batch b, heads h*8..h*8+7, plus its partial contribution
to the output projection (contraction over its 512 features). Host sums the
two partials per batch and stacks.

Kernel layout strategy (per core, all feature-major "transposed" tensors):
  xT   [1024,1500]  (host-pretransposed x[b].T)
  qT/kT = W^T.T @ xT accumulated over 8 state tiles -> [512,1500] feature-major
  v    [1500,512] natural layout, augmented with a ones column per head
  S^T  [k,q] per head computed as (kT tile).T @ qT chunk -> softmax along
       partitions never needed: exp on ACT (scores bounded, no max-sub),
       denominators from the ones column of v via the U matmul:
       U[65,q] = v_aug.T @ P^T, row 64 = sum_k P.
  wvT  [512,1500] = U[0:64]/U[64] per head -> stationary for out-projection.
  y_partial [1500,1024] = wvT.T @ woT (+bo on even cores only, via input data)
"""

import os
import numpy as np

N_STATE = 1024
B = 4
S = 1500
F = 512          # features per core (8 heads x 64)
NST = 8          # state k-tiles of 128 (contraction for projections)
NKT = 12         # seq k-tiles of 128 (attention contraction), last = 92
KPAD = 1536      # padded k extent (12*128)
QCH = [(0, 512), (512, 512), (1024, 476)]  # q chunks
VBLK = 520       # 8 heads * 65 cols (64 d + ones) per seq tile in v_sb
SCALE = 0.125    # 1/sqrt(64)
NCORES = 8

# matmul input dtype: "f32r" (full speed, reduced precision), "f32" (1/4 speed)
MM_MODE = os.environ.get("KERNEL_MM_MODE", "f32r")

_CACHE = {}
LAST_RESULTS = None


def _build(mm_mode: str):
    import concourse.bass as bass
    import concourse.mybir as mybir
    import concourse.tile as tile
    from concourse import bacc

    f32 = mybir.dt.float32
    Exp = mybir.ActivationFunctionType.Exp

    if mm_mode == "f32r":
        mdt = mybir.dt.float32r
    elif mm_mode == "f32":
        mdt = f32
    else:
        raise ValueError(mm_mode)

    nc = bacc.Bacc("TRN2", target_bir_lowering=False, debug=False,
                   num_devices=NCORES)

    xT = nc.dram_tensor("xT", [N_STATE, S], mdt, kind="ExternalInput").ap()
    wq = nc.dram_tensor("wq", [N_STATE, F], mdt, kind="ExternalInput").ap()
    wk = nc.dram_tensor("wk", [N_STATE, F], mdt, kind="ExternalInput").ap()
    wv = nc.dram_tensor("wv", [N_STATE, F], mdt, kind="ExternalInput").ap()
    wo = nc.dram_tensor("wo", [F, N_STATE], mdt, kind="ExternalInput").ap()
    bq = nc.dram_tensor("bq", [F], f32, kind="ExternalInput").ap()
    bv = nc.dram_tensor("bv", [F], f32, kind="ExternalInput").ap()
    vinit = nc.dram_tensor("vinit", [NKT * VBLK], mdt,
                           kind="ExternalInput").ap()
    y = nc.dram_tensor("y", [S, N_STATE], f32, kind="ExternalOutput").ap()
    # DRAM scratch for bouncing softmax denominators (partition-broadcast
    # DMA reads are only legal from DRAM). One slot per (head, qchunk).
    rs_dram = nc.dram_tensor("rs_dram", [24, 512], f32).ap()

    def mm(out, lhsT, rhs, **kw):
        nc.tensor.matmul(out=out, lhsT=lhsT, rhs=rhs, **kw)

    with tile.TileContext(nc) as tc:
        with (
            tc.tile_pool(name="sb", bufs=1) as sb,
            tc.tile_pool(name="sbw", bufs=3) as sbw,
            tc.tile_pool(name="ptp", bufs=2) as ptp,
            tc.tile_pool(name="sm", bufs=2) as sm,
            tc.tile_pool(name="ysp", bufs=3) as ysp,
            tc.tile_pool(name="ps", bufs=2, space="PSUM") as psp,
        ):
            # ---------------- persistent SBUF ----------------
            xT_sb = sb.tile([128, NST * S], mdt, name="xT_sb", tag="bigA")
            qT_sb = sb.tile([128, 4 * S], mdt, name="qT_sb", tag="qT")
            kT_sb = sb.tile([128, 4 * S], mdt, name="kT_sb", tag="kT")
            v_sb = sb.tile([128, NKT * VBLK], mdt, name="v_sb", tag="v")
            wv_sb = sb.tile([128, NST * F], mdt, name="wv_sb", tag="wfull")
            bq_sb = sb.tile([128, 4], f32, name="bq_sb", tag="bq")
            bv_sb = sb.tile([128, 4], f32, name="bv_sb", tag="bv")
            zero_col = sb.tile([128, 1], f32, name="zero_col", tag="z")

            # ---------------- input DMAs ----------------
            for st in range(NST):
                nc.sync.dma_start(
                    out=xT_sb[:, st * S:(st + 1) * S],
                    in_=xT[st * 128:(st + 1) * 128, :])
            nc.sync.dma_start(
                out=wv_sb[:].rearrange("p (s f) -> p s f", s=NST),
                in_=wv.rearrange("(s p) f -> p s f", p=128))
            nc.sync.dma_start(out=bq_sb[:],
                              in_=bq.rearrange("(f p) -> p f", p=128))
            nc.sync.dma_start(out=bv_sb[:],
                              in_=bv.rearrange("(f p) -> p f", p=128))

            nc.vector.memset(zero_col[:], 0.0)
            # ones for the v augmentation columns (f32r memset is not
            # encodable, so initialize the whole v tile from a host vector)
            nc.sync.dma_start(
                out=v_sb[:],
                in_=vinit[None, :].to_broadcast((128, NKT * VBLK)))

            # ---------------- phase 1a: q/k projections ----------------
            for wdram, dst, dstride, biased in (
                (wq, qT_sb, S, True),
                (wk, kT_sb, S, False),
            ):
                for ft in range(4):
                    wsl = sbw.tile([128, NST * 128], mdt, name="wsl",
                                   tag="wsl")
                    nc.sync.dma_start(
                        out=wsl[:].rearrange("p (s f) -> p s f", s=NST),
                        in_=wdram.rearrange("(s p) f -> p s f",
                                            p=128)[:, :, ft * 128:(ft + 1) * 128])
                    pacc = psp.tile([128, 1536], f32, name="pacc", tag="big3")
                    for q0, qn in QCH:
                        for st in range(NST):
                            mm(out=pacc[:, q0:q0 + qn],
                               lhsT=wsl[:, st * 128:(st + 1) * 128],
                               rhs=xT_sb[:, st * S + q0:st * S + q0 + qn],
                               start=(st == 0), stop=(st == NST - 1))
                    if biased:
                        nc.vector.tensor_scalar_add(
                            out=dst[:, ft * dstride:ft * dstride + S],
                            in0=pacc[:, 0:S],
                            scalar1=bq_sb[:, ft:ft + 1])
                    else:
                        nc.vector.tensor_scalar_add(
                            out=dst[:, ft * dstride:ft * dstride + S],
                            in0=pacc[:, 0:S],
                            scalar1=zero_col[:, 0:1])

            # ---------------- phase 1b: v projection ----------------
            for sq in range(NKT):
                sn = min(128, S - sq * 128)
                pv = psp.tile([128, 512], f32, name="pv", tag="acc")
                for st in range(NST):
                    mm(out=pv[0:sn, :],
                       lhsT=xT_sb[:, st * S + sq * 128:st * S + sq * 128 + sn],
                       rhs=wv_sb[:, st * F:(st + 1) * F],
                       start=(st == 0), stop=(st == NST - 1))
                for h in range(8):
                    nc.vector.tensor_scalar_add(
                        out=v_sb[0:sn, sq * VBLK + h * 65:
                                 sq * VBLK + h * 65 + 64],
                        in0=pv[0:sn, h * 64:(h + 1) * 64],
                        scalar1=zero_col[0:sn, 0:1])

            # ---------------- phase 2: attention ----------------
            # reuses the xT_sb slot (tag bigA): every read of xT_sb is in
            # phase 1; Tile inserts the WAR dependency.
            wvT_sb = sb.tile([128, 4 * S], mdt, name="wvT_sb", tag="bigA")
            for hp in range(4):
                for e in range(2):
                    head = hp * 2 + e
                    pb = e * 64
                    for qi, (q0, qn) in enumerate(QCH):
                        uacc = psp.tile([128, 512], f32, name="uacc",
                                        tag="acc")
                        for kg in range(4):
                            st_ps = psp.tile([128, 1536], f32, name="st_ps",
                                             tag="big3")
                            pt = ptp.tile([128, 1536], mdt, name="pt",
                                          tag="pt")
                            for j in range(3):
                                kt = kg * 3 + j
                                kn = min(128, S - kt * 128)
                                mm(out=st_ps[0:kn, j * 512:j * 512 + qn],
                                   lhsT=kT_sb[pb:pb + 64,
                                              hp * S + kt * 128:
                                              hp * S + kt * 128 + kn],
                                   rhs=qT_sb[pb:pb + 64,
                                             hp * S + q0:hp * S + q0 + qn])
                            def do_exp(rows, g0, g1):
                                if qn == 512:
                                    nc.scalar.activation(
                                        pt[0:rows, g0 * 512:g1 * 512],
                                        st_ps[0:rows, g0 * 512:g1 * 512],
                                        Exp, scale=SCALE)
                                else:
                                    nc.scalar.activation(
                                        pt[0:rows, g0 * 512:g1 * 512]
                                        .rearrange("p (g q) -> p g q",
                                                   g=g1 - g0)[:, :, 0:qn],
                                        st_ps[0:rows, g0 * 512:g1 * 512]
                                        .rearrange("p (g q) -> p g q",
                                                   g=g1 - g0)[:, :, 0:qn],
                                        Exp, scale=SCALE)
                            if kg < 3:
                                do_exp(128, 0, 3)
                            else:
                                do_exp(128, 0, 2)
                                do_exp(92, 2, 3)
                            for j in range(3):
                                kt = kg * 3 + j
                                kn = min(128, S - kt * 128)
                                mm(out=uacc[0:65, 0:qn],
                                   lhsT=v_sb[0:kn,
                                             kt * VBLK + head * 65:
                                             kt * VBLK + head * 65 + 65],
                                   rhs=pt[0:kn, j * 512:j * 512 + qn],
                                   start=(kt == 0), stop=(kt == NKT - 1))
                        slot = head * 3 + qi
                        rs = sm.tile([1, 512], f32, name="rs", tag="rs")
                        nc.vector.reciprocal(rs[0:1, 0:qn], uacc[64:65, 0:qn])
                        nc.sync.dma_start(out=rs_dram[slot:slot + 1, 0:qn],
                                          in_=rs[0:1, 0:qn])
                        rb = sm.tile([64, 512], f32, name="rb", tag="rb")
                        nc.sync.dma_start(
                            out=rb[0:64, 0:qn],
                            in_=rs_dram[slot:slot + 1,
                                        0:qn].to_broadcast((64, qn)))
                        nc.vector.tensor_mul(
                            wvT_sb[pb:pb + 64, hp * S + q0:hp * S + q0 + qn],
                            uacc[0:64, 0:qn],
                            rb[0:64, 0:qn])
                        nc.vector.tensor_scalar_add(
                            out=wvT_sb[pb:pb + 64,
                                       hp * S + q0:hp * S + q0 + qn],
                            in0=wvT_sb[pb:pb + 64,
                                       hp * S + q0:hp * S + q0 + qn],
                            scalar1=bv_sb[pb:pb + 64, hp:hp + 1])

            # ---------------- phase 3: output projection ----------------
            wo_sb = sb.tile([128, 4 * N_STATE], mdt, name="wo_sb", tag="wfull")
            nc.sync.dma_start(
                out=wo_sb[:].rearrange("p (h o) -> p h o", h=4),
                in_=wo.rearrange("(h p) o -> p h o", p=128))
            for sq in range(NKT):
                sn = min(128, S - sq * 128)
                for ch in range(2):
                    py_ = psp.tile([128, 512], f32, name="py_", tag="acc")
                    for hp in range(4):
                        mm(out=py_[0:sn, :],
                           lhsT=wvT_sb[:, hp * S + sq * 128:
                                       hp * S + sq * 128 + sn],
                           rhs=wo_sb[:, hp * N_STATE + ch * 512:
                                     hp * N_STATE + ch * 512 + 512],
                           start=(hp == 0), stop=(hp == 3))
                    yt = ysp.tile([128, 512], f32, name="yt", tag="yt")
                    nc.vector.tensor_scalar_add(
                        out=yt[0:sn, :], in0=py_[0:sn, :],
                        scalar1=zero_col[0:sn, 0:1])
                    nc.sync.dma_start(
                        out=y[sq * 128:sq * 128 + sn,
                              ch * 512:(ch + 1) * 512],
                        in_=yt[0:sn, :])

    nc.compile()
    return nc


def get_nc(mm_mode: str = MM_MODE):
    if mm_mode not in _CACHE:
        _CACHE[mm_mode] = _build(mm_mode)
    return _CACHE[mm_mode]


def make_in_maps(x, Wq, bq, Wk, Wv, bv, Wo, bo):
    x = np.asarray(x, dtype=np.float32)
    Wq = np.asarray(Wq, dtype=np.float32)
    Wk = np.asarray(Wk, dtype=np.float32)
    Wv = np.asarray(Wv, dtype=np.float32)
    Wo = np.asarray(Wo, dtype=np.float32)
    bq = np.asarray(bq, dtype=np.float32)
    bv = np.asarray(bv, dtype=np.float32)
    in_maps = []
    for c in range(NCORES):
        b, h = divmod(c, 2)
        sl = slice(h * F, (h + 1) * F)
        in_maps.append(dict(
            xT=np.ascontiguousarray(x[b].T),
            wq=np.ascontiguousarray(Wq[sl, :].T),
            wk=np.ascontiguousarray(Wk[sl, :].T),
            wv=np.ascontiguousarray(Wv[sl, :].T),
            wo=np.ascontiguousarray(Wo[:, sl].T),
            bq=np.ascontiguousarray(bq[sl]),
            bv=np.ascontiguousarray(bv[sl]),
            vinit=np.ones(NKT * VBLK, dtype=np.float32),
        ))
    return in_maps


def kernel(x, Wq, bq, Wk, Wv, bv, Wo, bo):
    global LAST_RESULTS
    from concourse.bass_utils import run_bass_kernel_spmd

    nc = get_nc()
    in_maps = make_in_maps(x, Wq, bq, Wk, Wv, bv, Wo, bo)
    res = run_bass_kernel_spmd(nc, in_maps, list(range(NCORES)))
    LAST_RESULTS = res
    bo32 = np.asarray(bo, dtype=np.float32)
    out = np.stack([res.results[2 * b]["y"] + res.results[2 * b + 1]["y"]
                    + bo32[None, :] for b in range(B)])
    return out.astype(np.float32)


# revision 23
# speedup vs baseline: 1.0929x; 1.0929x over previous
"""Multi-head attention (B=4, S=1500, D=1024, H=16) on 8 TRN2 NeuronCores.

Sharding: (batch, head-half) -> core c = 2*b + h; each core computes the
full attention for batch b, heads h*8..h*8+7, plus its partial contribution
to the output projection (contraction over its 512 features). Host sums the
two partials per batch and stacks.

Kernel layout strategy (per core, all feature-major "transposed" tensors):
  xT   [1024,1500]  (host-pretransposed x[b].T)
  qT/kT = W^T.T @ xT accumulated over 8 state tiles -> [512,1500] feature-major
  v    [1500,512] natural layout, augmented with a ones column per head
  S^T  [k,q] per head computed as (kT tile).T @ qT chunk -> softmax along
       partitions never needed: exp on ACT (scores bounded, no max-sub),
       denominators from the ones column of v via the U matmul:
       U[65,q] = v_aug.T @ P^T, row 64 = sum_k P.
  wvT  [512,1500] = U[0:64]/U[64] per head -> stationary for out-projection.
  y_partial [1500,1024] = wvT.T @ woT (+bo on even cores only, via input data)
"""

import os
import numpy as np

N_STATE = 1024
B = 4
S = 1500
F = 512          # features per core (8 heads x 64)
NST = 8          # state k-tiles of 128 (contraction for projections)
NKT = 12         # seq k-tiles of 128 (attention contraction), last = 92
KPAD = 1536      # padded k extent (12*128)
QCH = [(0, 512), (512, 512), (1024, 476)]  # q chunks
VBLK = 520       # 8 heads * 65 cols (64 d + ones) per seq tile in v_sb
SCALE = 0.125    # 1/sqrt(64)
NCORES = 8

# matmul input dtype: "f32r" (full speed, reduced precision), "f32" (1/4 speed)
MM_MODE = os.environ.get("KERNEL_MM_MODE", "f32r")

_CACHE = {}
LAST_RESULTS = None


def _build(mm_mode: str):
    import concourse.bass as bass
    import concourse.mybir as mybir
    import concourse.tile as tile
    from concourse import bacc

    f32 = mybir.dt.float32
    Exp = mybir.ActivationFunctionType.Exp

    if mm_mode == "f32r":
        mdt = mybir.dt.float32r
    elif mm_mode == "f32":
        mdt = f32
    else:
        raise ValueError(mm_mode)

    nc = bacc.Bacc("TRN2", target_bir_lowering=False, debug=False,
                   num_devices=NCORES)

    xT = nc.dram_tensor("xT", [N_STATE, S], mdt, kind="ExternalInput").ap()
    wq = nc.dram_tensor("wq", [N_STATE, F], mdt, kind="ExternalInput").ap()
    wk = nc.dram_tensor("wk", [N_STATE, F], mdt, kind="ExternalInput").ap()
    wv = nc.dram_tensor("wv", [N_STATE, F], mdt, kind="ExternalInput").ap()
    wo = nc.dram_tensor("wo", [F, N_STATE], mdt, kind="ExternalInput").ap()
    bq = nc.dram_tensor("bq", [F], f32, kind="ExternalInput").ap()
    bv = nc.dram_tensor("bv", [F], f32, kind="ExternalInput").ap()
    vinit = nc.dram_tensor("vinit", [NKT * VBLK], mdt,
                           kind="ExternalInput").ap()
    y = nc.dram_tensor("y", [S, N_STATE], f32, kind="ExternalOutput").ap()
    # DRAM scratch for bouncing softmax denominators (partition-broadcast
    # DMA reads are only legal from DRAM). One slot per (head, qchunk).
    rs_dram = nc.dram_tensor("rs_dram", [24, 512], f32).ap()

    def mm(out, lhsT, rhs, **kw):
        nc.tensor.matmul(out=out, lhsT=lhsT, rhs=rhs, **kw)

    with tile.TileContext(nc) as tc:
        with (
            tc.tile_pool(name="sb", bufs=1) as sb,
            tc.tile_pool(name="sbw", bufs=3) as sbw,
            tc.tile_pool(name="ptp", bufs=4) as ptp,
            tc.tile_pool(name="sm", bufs=3) as sm,
            tc.tile_pool(name="ysp", bufs=3) as ysp,
            tc.tile_pool(name="ps", bufs=2, space="PSUM") as psp,
        ):
            # ---------------- persistent SBUF ----------------
            xT_sb = sb.tile([128, NST * S], mdt, name="xT_sb", tag="bigA")
            qT_sb = sb.tile([128, 4 * S], mdt, name="qT_sb", tag="qT")
            kT_sb = sb.tile([128, 4 * S], mdt, name="kT_sb", tag="kT")
            v_sb = sb.tile([128, NKT * VBLK], mdt, name="v_sb", tag="v")
            wv_sb = sb.tile([128, NST * F], mdt, name="wv_sb", tag="wfull")
            bq_sb = sb.tile([128, 4], f32, name="bq_sb", tag="bq")
            bv_sb = sb.tile([128, 4], f32, name="bv_sb", tag="bv")
            zero_col = sb.tile([128, 1], f32, name="zero_col", tag="z")

            # ---------------- input DMAs ----------------
            # the first projection tile only needs its weight slice and the
            # x tiles, in contraction order -- emit those first so PE can
            # start within a few us instead of waiting behind the bulk loads
            wsl0 = sbw.tile([128, NST * 128], mdt, name="wsl0", tag="wsl")
            nc.sync.dma_start(
                out=wsl0[:].rearrange("p (s f) -> p s f", s=NST),
                in_=wq.rearrange("(s p) f -> p s f", p=128)[:, :, 0:128])
            for st in range(NST):
                nc.sync.dma_start(
                    out=xT_sb[:, st * S:(st + 1) * S],
                    in_=xT[st * 128:(st + 1) * 128, :])
            nc.sync.dma_start(out=bq_sb[:],
                              in_=bq.rearrange("(f p) -> p f", p=128))
            nc.sync.dma_start(out=bv_sb[:],
                              in_=bv.rearrange("(f p) -> p f", p=128))
            nc.vector.memset(zero_col[:], 0.0)

            # ---------------- phase 1a: q/k projections ----------------
            for wdram, dst, dstride, biased in (
                (wq, qT_sb, S, True),
                (wk, kT_sb, S, False),
            ):
                for ft in range(4):
                    if wdram is wq and ft == 0:
                        wsl = wsl0
                    else:
                        wsl = sbw.tile([128, NST * 128], mdt, name="wsl",
                                       tag="wsl")
                        nc.sync.dma_start(
                            out=wsl[:].rearrange("p (s f) -> p s f", s=NST),
                            in_=wdram.rearrange(
                                "(s p) f -> p s f",
                                p=128)[:, :, ft * 128:(ft + 1) * 128])
                    pacc = psp.tile([128, 1536], f32, name="pacc", tag="big3")
                    for q0, qn in QCH:
                        for st in range(NST):
                            mm(out=pacc[:, q0:q0 + qn],
                               lhsT=wsl[:, st * 128:(st + 1) * 128],
                               rhs=xT_sb[:, st * S + q0:st * S + q0 + qn],
                               start=(st == 0), stop=(st == NST - 1))
                    if biased:
                        nc.vector.tensor_scalar_add(
                            out=dst[:, ft * dstride:ft * dstride + S],
                            in0=pacc[:, 0:S],
                            scalar1=bq_sb[:, ft:ft + 1])
                    else:
                        nc.vector.tensor_scalar_add(
                            out=dst[:, ft * dstride:ft * dstride + S],
                            in0=pacc[:, 0:S],
                            scalar1=zero_col[:, 0:1])

            # ---------------- phase 1b: v projection ----------------
            nc.sync.dma_start(
                out=wv_sb[:].rearrange("p (s f) -> p s f", s=NST),
                in_=wv.rearrange("(s p) f -> p s f", p=128))
            # ones for the v augmentation columns (f32r memset is not
            # encodable, so initialize the whole v tile from a host vector)
            nc.sync.dma_start(
                out=v_sb[:],
                in_=vinit[None, :].to_broadcast((128, NKT * VBLK)))
            for sq in range(NKT):
                sn = min(128, S - sq * 128)
                pv = psp.tile([128, 512], f32, name="pv", tag="acc")
                for st in range(NST):
                    mm(out=pv[0:sn, :],
                       lhsT=xT_sb[:, st * S + sq * 128:st * S + sq * 128 + sn],
                       rhs=wv_sb[:, st * F:(st + 1) * F],
                       start=(st == 0), stop=(st == NST - 1))
                for h in range(8):
                    nc.vector.tensor_scalar_add(
                        out=v_sb[0:sn, sq * VBLK + h * 65:
                                 sq * VBLK + h * 65 + 64],
                        in0=pv[0:sn, h * 64:(h + 1) * 64],
                        scalar1=zero_col[0:sn, 0:1])

            # ---------------- phase 2: attention ----------------
            # reuses the xT_sb slot (tag bigA): every read of xT_sb is in
            # phase 1; Tile inserts the WAR dependency.
            wvT_sb = sb.tile([128, 4 * S], mdt, name="wvT_sb", tag="bigA")
            wo_sb = sb.tile([128, 4 * N_STATE], mdt, name="wo_sb", tag="wfull")
            nc.sync.dma_start(
                out=wo_sb[:].rearrange("p (h o) -> p h o", h=4),
                in_=wo.rearrange("(h p) o -> p h o", p=128))

            def attention_pair(hp, qi, q0, qn):
                """Both heads of the pair, interleaved: the S^T matmuls use
                only 64 PE rows (tile positions 0 / 64), so A/B can occupy
                the array simultaneously; U lags one k-group so PE never
                waits on ACT."""
                uacc = [psp.tile([128, 512], f32, name=f"uacc{e}",
                                 tag="acc") for e in range(2)]

                def do_st(kg, e):
                    pb = e * 64
                    st_ps = psp.tile([128, 1536], f32, name="st_ps",
                                     tag="big3")
                    pt = ptp.tile([128, 1536], mdt, name="pt", tag="pt")
                    for j in range(3):
                        kt = kg * 3 + j
                        kn = min(128, S - kt * 128)
                        mm(out=st_ps[0:kn, j * 512:j * 512 + qn],
                           lhsT=kT_sb[pb:pb + 64,
                                      hp * S + kt * 128:
                                      hp * S + kt * 128 + kn],
                           rhs=qT_sb[pb:pb + 64,
                                     hp * S + q0:hp * S + q0 + qn])

                    def do_exp(rows, g0, g1):
                        if qn == 512:
                            nc.scalar.activation(
                                pt[0:rows, g0 * 512:g1 * 512],
                                st_ps[0:rows, g0 * 512:g1 * 512],
                                Exp, scale=SCALE)
                        else:
                            nc.scalar.activation(
                                pt[0:rows, g0 * 512:g1 * 512]
                                .rearrange("p (g q) -> p g q",
                                           g=g1 - g0)[:, :, 0:qn],
                                st_ps[0:rows, g0 * 512:g1 * 512]
                                .rearrange("p (g q) -> p g q",
                                           g=g1 - g0)[:, :, 0:qn],
                                Exp, scale=SCALE)
                    if kg < 3:
                        do_exp(128, 0, 3)
                    else:
                        do_exp(128, 0, 2)
                        do_exp(92, 2, 3)
                    return pt

                def do_u(kg, e, pt):
                    head = hp * 2 + e
                    for j in range(3):
                        kt = kg * 3 + j
                        kn = min(128, S - kt * 128)
                        mm(out=uacc[e][0:65, 0:qn],
                           lhsT=v_sb[0:kn,
                                     kt * VBLK + head * 65:
                                     kt * VBLK + head * 65 + 65],
                           rhs=pt[0:kn, j * 512:j * 512 + qn],
                           start=(kt == 0), stop=(kt == NKT - 1))

                pts = {}
                for kg in range(4):
                    for e in range(2):
                        pts[(kg, e)] = do_st(kg, e)
                    if kg > 0:
                        for e in range(2):
                            do_u(kg - 1, e, pts.pop((kg - 1, e)))
                for e in range(2):
                    do_u(3, e, pts.pop((3, e)))

                for e in range(2):
                    head = hp * 2 + e
                    pb = e * 64
                    # stage U out of PSUM so the accumulators free quickly
                    usb = sm.tile([65, 512], f32, name="usb", tag="usb",
                                  bufs=4)
                    nc.vector.tensor_scalar_add(
                        out=usb[0:65, 0:qn], in0=uacc[e][0:65, 0:qn],
                        scalar1=zero_col[0:65, 0:1])
                    slot = head * 3 + qi
                    rs = sm.tile([1, 512], f32, name="rs", tag="rs", bufs=4)
                    nc.vector.reciprocal(rs[0:1, 0:qn], usb[64:65, 0:qn])
                    nc.sync.dma_start(out=rs_dram[slot:slot + 1, 0:qn],
                                      in_=rs[0:1, 0:qn])
                    rb = sm.tile([64, 512], f32, name="rb", tag="rb", bufs=4)
                    nc.sync.dma_start(
                        out=rb[0:64, 0:qn],
                        in_=rs_dram[slot:slot + 1,
                                    0:qn].to_broadcast((64, qn)))
                    nc.vector.tensor_mul(
                        wvT_sb[pb:pb + 64, hp * S + q0:hp * S + q0 + qn],
                        usb[0:64, 0:qn],
                        rb[0:64, 0:qn])
                    nc.vector.tensor_scalar_add(
                        out=wvT_sb[pb:pb + 64,
                                   hp * S + q0:hp * S + q0 + qn],
                        in0=wvT_sb[pb:pb + 64,
                                   hp * S + q0:hp * S + q0 + qn],
                        scalar1=bv_sb[pb:pb + 64, hp:hp + 1])

            def out_proj(sq):
                sn = min(128, S - sq * 128)
                for ch in range(2):
                    py_ = psp.tile([128, 512], f32, name="py_", tag="acc")
                    for hp in range(4):
                        mm(out=py_[0:sn, :],
                           lhsT=wvT_sb[:, hp * S + sq * 128:
                                       hp * S + sq * 128 + sn],
                           rhs=wo_sb[:, hp * N_STATE + ch * 512:
                                     hp * N_STATE + ch * 512 + 512],
                           start=(hp == 0), stop=(hp == 3))
                    yt = ysp.tile([128, 512], f32, name="yt", tag="yt")
                    nc.vector.tensor_scalar_add(
                        out=yt[0:sn, :], in0=py_[0:sn, :],
                        scalar1=zero_col[0:sn, 0:1])
                    nc.sync.dma_start(
                        out=y[sq * 128:sq * 128 + sn,
                              ch * 512:(ch + 1) * 512],
                        in_=yt[0:sn, :])

            # q-chunk outermost; the output projection for chunk i is
            # emitted after the attention of chunk i+1 so PE never stalls
            # on the normalize chains of the chunk it projects.
            for qi, (q0, qn) in enumerate(QCH):
                for hp in range(4):
                    attention_pair(hp, qi, q0, qn)
                if qi >= 1:
                    for sq in range(4 * (qi - 1), 4 * qi):
                        out_proj(sq)
            for sq in range(8, NKT):
                out_proj(sq)

    nc.compile()
    return nc


def get_nc(mm_mode: str = MM_MODE):
    if mm_mode not in _CACHE:
        _CACHE[mm_mode] = _build(mm_mode)
    return _CACHE[mm_mode]


def make_in_maps(x, Wq, bq, Wk, Wv, bv, Wo, bo):
    x = np.asarray(x, dtype=np.float32)
    Wq = np.asarray(Wq, dtype=np.float32)
    Wk = np.asarray(Wk, dtype=np.float32)
    Wv = np.asarray(Wv, dtype=np.float32)
    Wo = np.asarray(Wo, dtype=np.float32)
    bq = np.asarray(bq, dtype=np.float32)
    bv = np.asarray(bv, dtype=np.float32)
    in_maps = []
    for c in range(NCORES):
        b, h = divmod(c, 2)
        sl = slice(h * F, (h + 1) * F)
        in_maps.append(dict(
            xT=np.ascontiguousarray(x[b].T),
            wq=np.ascontiguousarray(Wq[sl, :].T),
            wk=np.ascontiguousarray(Wk[sl, :].T),
            wv=np.ascontiguousarray(Wv[sl, :].T),
            wo=np.ascontiguousarray(Wo[:, sl].T),
            bq=np.ascontiguousarray(bq[sl]),
            bv=np.ascontiguousarray(bv[sl]),
            vinit=np.ones(NKT * VBLK, dtype=np.float32),
        ))
    return in_maps


def kernel(x, Wq, bq, Wk, Wv, bv, Wo, bo):
    global LAST_RESULTS
    from concourse.bass_utils import run_bass_kernel_spmd

    nc = get_nc()
    in_maps = make_in_maps(x, Wq, bq, Wk, Wv, bv, Wo, bo)
    res = run_bass_kernel_spmd(nc, in_maps, list(range(NCORES)))
    LAST_RESULTS = res
    bo32 = np.asarray(bo, dtype=np.float32)
    out = np.stack([res.results[2 * b]["y"] + res.results[2 * b + 1]["y"]
                    + bo32[None, :] for b in range(B)])
    return out.astype(np.float32)


# revision 28
# speedup vs baseline: 1.1406x; 1.0436x over previous
"""Multi-head attention (B=4, S=1500, D=1024, H=16) on 8 TRN2 NeuronCores.

Sharding: (batch, head-half) -> core c = 2*b + h; each core computes the
full attention for batch b, heads h*8..h*8+7, plus its partial contribution
to the output projection (contraction over its 512 features). Host sums the
two partials per batch and stacks.

Kernel layout strategy (per core, all feature-major "transposed" tensors):
  xT   [1024,1500]  (host-pretransposed x[b].T)
  qT/kT = W^T.T @ xT accumulated over 8 state tiles -> [512,1500] feature-major
  v    [1500,512] natural layout, augmented with a ones column per head
  S^T  [k,q] per head computed as (kT tile).T @ qT chunk -> softmax along
       partitions never needed: exp on ACT (scores bounded, no max-sub),
       denominators from the ones column of v via the U matmul:
       U[65,q] = v_aug.T @ P^T, row 64 = sum_k P.
  wvT  [512,1500] = U[0:64]/U[64] + bv per head -> out-projection stationary.
  y_partial [1500,1024] = wvT.T @ woT; host adds bo while summing the pair.

All matmul inputs are tagged float32r (full-rate PE at ~TF32-like precision,
measured end-to-end rel err ~3e-4). Attention runs per head-pair with the
two heads' S^T matmuls on PE row-tile positions 0/64 (64-row contraction),
and the U matmuls lag one k-group behind exp so PE never waits on ACT.
"""

import os
import numpy as np

N_STATE = 1024
B = 4
S = 1500
F = 512          # features per core (8 heads x 64)
NST = 8          # state k-tiles of 128 (contraction for projections)
NKT = 12         # seq k-tiles of 128 (attention contraction), last = 92
KPAD = 1536      # padded k extent (12*128)
QCH = [(0, 512), (512, 512), (1024, 476)]  # q chunks
VBLK = 520       # 8 heads * 65 cols (64 d + ones) per seq tile in v_sb
SCALE = 0.125    # 1/sqrt(64)
NCORES = 8

# matmul input dtype: "f32r" (full speed, reduced precision), "f32" (1/4 speed)
MM_MODE = os.environ.get("KERNEL_MM_MODE", "f32r")

_CACHE = {}
LAST_RESULTS = None


def _build(mm_mode: str):
    import concourse.bass as bass
    import concourse.mybir as mybir
    import concourse.tile as tile
    from concourse import bacc

    f32 = mybir.dt.float32
    Exp = mybir.ActivationFunctionType.Exp

    if mm_mode == "f32r":
        mdt = mybir.dt.float32r
    elif mm_mode == "f32":
        mdt = f32
    else:
        raise ValueError(mm_mode)

    nc = bacc.Bacc("TRN2", target_bir_lowering=False, debug=False,
                   num_devices=NCORES)

    xT = nc.dram_tensor("xT", [N_STATE, S], mdt, kind="ExternalInput").ap()
    wq = nc.dram_tensor("wq", [N_STATE, F], mdt, kind="ExternalInput").ap()
    wk = nc.dram_tensor("wk", [N_STATE, F], mdt, kind="ExternalInput").ap()
    wv = nc.dram_tensor("wv", [N_STATE, F], mdt, kind="ExternalInput").ap()
    wo = nc.dram_tensor("wo", [F, N_STATE], mdt, kind="ExternalInput").ap()
    bq = nc.dram_tensor("bq", [F], f32, kind="ExternalInput").ap()
    bv = nc.dram_tensor("bv", [F], f32, kind="ExternalInput").ap()
    vinit = nc.dram_tensor("vinit", [NKT * VBLK], mdt,
                           kind="ExternalInput").ap()
    zinit = nc.dram_tensor("zinit", [4 * (KPAD - S)], mdt,
                           kind="ExternalInput").ap()
    y = nc.dram_tensor("y", [S, N_STATE], f32, kind="ExternalOutput").ap()
    # DRAM scratch for bouncing softmax denominators (partition-broadcast
    # DMA reads are only legal from DRAM). One slot per (head, qchunk).
    rs_dram = nc.dram_tensor("rs_dram", [24, 512], f32).ap()

    def mm(out, lhsT, rhs, **kw):
        nc.tensor.matmul(out=out, lhsT=lhsT, rhs=rhs, **kw)

    with tile.TileContext(nc) as tc:
        with (
            tc.tile_pool(name="sb", bufs=1) as sb,
            tc.tile_pool(name="sbw", bufs=3) as sbw,
            tc.tile_pool(name="ptp", bufs=4) as ptp,
            tc.tile_pool(name="sm", bufs=3) as sm,
            tc.tile_pool(name="ysp", bufs=3) as ysp,
            tc.tile_pool(name="ps", bufs=2, space="PSUM") as psp,
        ):
            # ---------------- persistent SBUF ----------------
            xT_sb = sb.tile([128, NST * S], mdt, name="xT_sb", tag="bigA")
            qT_sb = sb.tile([128, 4 * S], mdt, name="qT_sb", tag="qT")
            kT_sb = sb.tile([128, 4 * KPAD], mdt, name="kT_sb", tag="kT")
            v_sb = sb.tile([128, NKT * VBLK], mdt, name="v_sb", tag="v")
            wv_sb = sb.tile([128, NST * F], mdt, name="wv_sb", tag="wfull")
            bq_sb = sb.tile([128, 4], f32, name="bq_sb", tag="bq")
            bv_sb = sb.tile([128, 4], f32, name="bv_sb", tag="bv")
            zero_col = sb.tile([128, 1], f32, name="zero_col", tag="z")

            # ---------------- input DMAs ----------------
            # the first projection tile only needs its weight slice and the
            # x tiles, in contraction order -- emit those first so PE can
            # start within a few us instead of waiting behind the bulk loads
            wsl0 = sbw.tile([128, NST * 128], mdt, name="wsl0", tag="wsl")
            for st in range(NST):
                nc.sync.dma_start(
                    out=wsl0[:, st * 128:(st + 1) * 128],
                    in_=wq[st * 128:(st + 1) * 128, 0:128])
                nc.sync.dma_start(
                    out=xT_sb[:, st * S:(st + 1) * S],
                    in_=xT[st * 128:(st + 1) * 128, :])
            nc.sync.dma_start(out=bq_sb[:],
                              in_=bq.rearrange("(f p) -> p f", p=128))
            nc.sync.dma_start(out=bv_sb[:],
                              in_=bv.rearrange("(f p) -> p f", p=128))
            nc.vector.memset(zero_col[:], 0.0)
            nc.sync.dma_start(
                out=kT_sb[:].rearrange("p (f c) -> p f c",
                                       f=4)[:, :, S:KPAD],
                in_=zinit.rearrange("(f c) -> f c",
                                    f=4)[None].to_broadcast(
                    (128, 4, KPAD - S)))

            # ---------------- phase 1a: q/k projections ----------------
            for wdram, dst, dstride, biased in (
                (wq, qT_sb, S, True),
                (wk, kT_sb, KPAD, False),
            ):
                for ft in range(4):
                    if wdram is wq and ft == 0:
                        wsl = wsl0
                    else:
                        wsl = sbw.tile([128, NST * 128], mdt, name="wsl",
                                       tag="wsl")
                        nc.sync.dma_start(
                            out=wsl[:].rearrange("p (s f) -> p s f", s=NST),
                            in_=wdram.rearrange(
                                "(s p) f -> p s f",
                                p=128)[:, :, ft * 128:(ft + 1) * 128])
                    pacc = psp.tile([128, 1536], f32, name="pacc", tag="big3")
                    for q0, qn in QCH:
                        for st in range(NST):
                            mm(out=pacc[:, q0:q0 + qn],
                               lhsT=wsl[:, st * 128:(st + 1) * 128],
                               rhs=xT_sb[:, st * S + q0:st * S + q0 + qn],
                               start=(st == 0), stop=(st == NST - 1))
                    if biased:
                        nc.vector.tensor_scalar_add(
                            out=dst[:, ft * dstride:ft * dstride + S],
                            in0=pacc[:, 0:S],
                            scalar1=bq_sb[:, ft:ft + 1])
                    else:
                        nc.vector.tensor_scalar_add(
                            out=dst[:, ft * dstride:ft * dstride + S],
                            in0=pacc[:, 0:S],
                            scalar1=zero_col[:, 0:1])

            # ---------------- phase 1b: v projection ----------------
            nc.sync.dma_start(
                out=wv_sb[:].rearrange("p (s f) -> p s f", s=NST),
                in_=wv.rearrange("(s p) f -> p s f", p=128))
            # ones for the v augmentation columns (f32r memset is not
            # encodable, so initialize the whole v tile from a host vector)
            nc.sync.dma_start(
                out=v_sb[:],
                in_=vinit[None, :].to_broadcast((128, NKT * VBLK)))
            for sq in range(NKT):
                sn = min(128, S - sq * 128)
                pv = psp.tile([128, 512], f32, name="pv", tag="acc")
                for st in range(NST):
                    mm(out=pv[0:sn, :],
                       lhsT=xT_sb[:, st * S + sq * 128:st * S + sq * 128 + sn],
                       rhs=wv_sb[:, st * F:(st + 1) * F],
                       start=(st == 0), stop=(st == NST - 1))
                for h in range(8):
                    nc.vector.tensor_scalar_add(
                        out=v_sb[0:sn, sq * VBLK + h * 65:
                                 sq * VBLK + h * 65 + 64],
                        in0=pv[0:sn, h * 64:(h + 1) * 64],
                        scalar1=zero_col[0:sn, 0:1])

            # ---------------- phase 2: attention ----------------
            # reuses the xT_sb slot (tag bigA): every read of xT_sb is in
            # phase 1; Tile inserts the WAR dependency.
            wvT_sb = sb.tile([128, 4 * S], mdt, name="wvT_sb", tag="bigA")
            wo_sb = sb.tile([128, 4 * N_STATE], mdt, name="wo_sb", tag="wfull")
            nc.sync.dma_start(
                out=wo_sb[:].rearrange("p (h o) -> p h o", h=4),
                in_=wo.rearrange("(h p) o -> p h o", p=128))

            def attention_pair(hp, qi, q0, qn):
                """Both heads of the pair, interleaved: the S^T matmuls use
                only 64 PE rows (tile positions 0 / 64), so A/B can occupy
                the array simultaneously; U lags one k-group so PE never
                waits on ACT."""
                uacc = [psp.tile([128, 512], f32, name=f"uacc{e}",
                                 tag="acc") for e in range(2)]

                def do_st(kg, e):
                    pb = e * 64
                    st_ps = psp.tile([128, 1536], f32, name="st_ps",
                                     tag="big3")
                    pt = ptp.tile([128, 1536], mdt, name="pt", tag="pt")
                    for j in range(3):
                        kt = kg * 3 + j
                        mm(out=st_ps[:, j * 512:j * 512 + qn],
                           lhsT=kT_sb[pb:pb + 64,
                                      hp * KPAD + kt * 128:
                                      hp * KPAD + (kt + 1) * 128],
                           rhs=qT_sb[pb:pb + 64,
                                     hp * S + q0:hp * S + q0 + qn])

                    def do_exp(rows, g0, g1):
                        if qn == 512:
                            nc.scalar.activation(
                                pt[0:rows, g0 * 512:g1 * 512],
                                st_ps[0:rows, g0 * 512:g1 * 512],
                                Exp, scale=SCALE)
                        else:
                            nc.scalar.activation(
                                pt[0:rows, g0 * 512:g1 * 512]
                                .rearrange("p (g q) -> p g q",
                                           g=g1 - g0)[:, :, 0:qn],
                                st_ps[0:rows, g0 * 512:g1 * 512]
                                .rearrange("p (g q) -> p g q",
                                           g=g1 - g0)[:, :, 0:qn],
                                Exp, scale=SCALE)
                    do_exp(128, 0, 3)
                    return pt

                def do_u(kg, e, pt):
                    head = hp * 2 + e
                    for j in range(3):
                        kt = kg * 3 + j
                        kn = min(128, S - kt * 128)
                        mm(out=uacc[e][0:65, 0:qn],
                           lhsT=v_sb[0:kn,
                                     kt * VBLK + head * 65:
                                     kt * VBLK + head * 65 + 65],
                           rhs=pt[0:kn, j * 512:j * 512 + qn],
                           start=(kt == 0), stop=(kt == NKT - 1))

                pts = {}
                for kg in range(4):
                    for e in range(2):
                        pts[(kg, e)] = do_st(kg, e)
                    if kg > 0:
                        for e in range(2):
                            do_u(kg - 1, e, pts.pop((kg - 1, e)))
                for e in range(2):
                    do_u(3, e, pts.pop((3, e)))

                for e in range(2):
                    head = hp * 2 + e
                    pb = e * 64
                    # stage U out of PSUM so the accumulators free quickly
                    usb = sm.tile([65, 512], f32, name="usb", tag="usb",
                                  bufs=4)
                    nc.vector.tensor_scalar_add(
                        out=usb[0:65, 0:qn], in0=uacc[e][0:65, 0:qn],
                        scalar1=zero_col[0:65, 0:1])
                    slot = head * 3 + qi
                    rs = sm.tile([1, 512], f32, name="rs", tag="rs", bufs=4)
                    nc.vector.reciprocal(rs[0:1, 0:qn], usb[64:65, 0:qn])
                    nc.sync.dma_start(out=rs_dram[slot:slot + 1, 0:qn],
                                      in_=rs[0:1, 0:qn])
                    rb = sm.tile([64, 512], f32, name="rb", tag="rb", bufs=4)
                    nc.sync.dma_start(
                        out=rb[0:64, 0:qn],
                        in_=rs_dram[slot:slot + 1,
                                    0:qn].to_broadcast((64, qn)))
                    nc.vector.tensor_mul(
                        wvT_sb[pb:pb + 64, hp * S + q0:hp * S + q0 + qn],
                        usb[0:64, 0:qn],
                        rb[0:64, 0:qn])
                    nc.vector.tensor_scalar_add(
                        out=wvT_sb[pb:pb + 64,
                                   hp * S + q0:hp * S + q0 + qn],
                        in0=wvT_sb[pb:pb + 64,
                                   hp * S + q0:hp * S + q0 + qn],
                        scalar1=bv_sb[pb:pb + 64, hp:hp + 1])

            def out_proj(sq):
                sn = min(128, S - sq * 128)
                for ch in range(2):
                    py_ = psp.tile([128, 512], f32, name="py_", tag="acc")
                    for hp in range(4):
                        mm(out=py_[0:sn, :],
                           lhsT=wvT_sb[:, hp * S + sq * 128:
                                       hp * S + sq * 128 + sn],
                           rhs=wo_sb[:, hp * N_STATE + ch * 512:
                                     hp * N_STATE + ch * 512 + 512],
                           start=(hp == 0), stop=(hp == 3))
                    yt = ysp.tile([128, 512], f32, name="yt", tag="yt")
                    nc.vector.tensor_scalar_add(
                        out=yt[0:sn, :], in0=py_[0:sn, :],
                        scalar1=zero_col[0:sn, 0:1])
                    nc.sync.dma_start(
                        out=y[sq * 128:sq * 128 + sn,
                              ch * 512:(ch + 1) * 512],
                        in_=yt[0:sn, :])

            # q-chunk outermost; the output projection for chunk i is
            # emitted after the attention of chunk i+1 so PE never stalls
            # on the normalize chains of the chunk it projects.
            for qi, (q0, qn) in enumerate(QCH):
                for hp in range(4):
                    attention_pair(hp, qi, q0, qn)
                if qi >= 1:
                    for sq in range(4 * (qi - 1), 4 * qi):
                        out_proj(sq)
            for sq in range(8, NKT):
                out_proj(sq)

    nc.compile()
    return nc


def get_nc(mm_mode: str = MM_MODE):
    if mm_mode not in _CACHE:
        _CACHE[mm_mode] = _build(mm_mode)
    return _CACHE[mm_mode]


def make_in_maps(x, Wq, bq, Wk, Wv, bv, Wo, bo):
    x = np.asarray(x, dtype=np.float32)
    Wq = np.asarray(Wq, dtype=np.float32)
    Wk = np.asarray(Wk, dtype=np.float32)
    Wv = np.asarray(Wv, dtype=np.float32)
    Wo = np.asarray(Wo, dtype=np.float32)
    bq = np.asarray(bq, dtype=np.float32)
    bv = np.asarray(bv, dtype=np.float32)
    in_maps = []
    for c in range(NCORES):
        b, h = divmod(c, 2)
        sl = slice(h * F, (h + 1) * F)
        in_maps.append(dict(
            xT=np.ascontiguousarray(x[b].T),
            wq=np.ascontiguousarray(Wq[sl, :].T),
            wk=np.ascontiguousarray(Wk[sl, :].T),
            wv=np.ascontiguousarray(Wv[sl, :].T),
            wo=np.ascontiguousarray(Wo[:, sl].T),
            bq=np.ascontiguousarray(bq[sl]),
            bv=np.ascontiguousarray(bv[sl]),
            vinit=np.ones(NKT * VBLK, dtype=np.float32),
            zinit=np.zeros(4 * (KPAD - S), dtype=np.float32),
        ))
    return in_maps


def kernel(x, Wq, bq, Wk, Wv, bv, Wo, bo):
    global LAST_RESULTS
    from concourse.bass_utils import run_bass_kernel_spmd

    nc = get_nc()
    in_maps = make_in_maps(x, Wq, bq, Wk, Wv, bv, Wo, bo)
    res = run_bass_kernel_spmd(nc, in_maps, list(range(NCORES)))
    LAST_RESULTS = res
    bo32 = np.asarray(bo, dtype=np.float32)
    out = np.stack([res.results[2 * b]["y"] + res.results[2 * b + 1]["y"]
                    + bo32[None, :] for b in range(B)])
    return out.astype(np.float32)
